# revision 25
# baseline (speedup 1.0000x reference)
"""BiGRU (S=512, B=64, I=256, H=512, L=2) Trainium2 Bass kernel (v2b).

Strategy: 4-way batch split x 2-way direction split across 8 NeuronCores.
Cores 0-3 run the forward GRU chain for batch quarters 0-3; cores 4-7 the
backward chain (fed time-reversed input, so the device program is
identical on every core).

v2b: the input projections (P phases) no longer run as separate phases
writing gx to DRAM.  Instead gx lives in an SBUF ring of RING chunks
(32 steps each) and the projection matmuls for chunk c+2 are interleaved
into the scan steps of chunk c ("fills"), executing in the PE-idle gap
while the per-step gate chain runs.  This keeps the tensor engine
continuously busy (so the HAM clock un-throttles 1.2 -> 2.4 GHz) and
removes the gx DRAM round trip.

Scan step schedule: PSUM groups in order r, n, z so the long n-gate
consumer chain (tn -> tn2 -> tanh -> blend) starts right at the end of
the matmul burst.  Blend uses scalar_tensor_tensor:
  h' = p1 - (z-1)*n  ==  z*h_prev + (1-z)*n.

Layer exchange: chunked pairwise AllGathers fired while S0 still runs
(y0ex fills from high columns down, so high chunks gather early); the
layer-1 fills read the partner half straight out of the gathered buffer
under an If/Else on pair rank.  Final un-transpose / un-reverse of the
output happens on the host.
"""

import os
import sys
import numpy as np

for _p in ("/opt/trn_rl_repo", "/root/.axon_site/_ro/trn_rl_repo"):
    if os.path.isdir(_p) and _p not in sys.path:
        sys.path.insert(0, _p)

import ml_dtypes
from contextlib import ExitStack

import concourse.bass as bass
import concourse.tile as tile
from concourse import bacc, mybir
from concourse.bass import ts
from concourse.bass_utils import run_bass_kernel_spmd

BF16 = mybir.dt.bfloat16
F32 = mybir.dt.float32
AF = mybir.ActivationFunctionType
ALU = mybir.AluOpType

S, B, I, H, L = 512, 64, 256, 512, 2
G = 3 * H            # 1536 gate rows (r, z, n)
NCORE = 8
BQ = B // 4          # 16 batch per core
SB = S * BQ          # 8192 moving columns
F = H // 128         # 4 h-fold chunks
M12 = G // 128       # 12 gate chunks
KI0 = I // 128       # 2 contraction chunks, layer-0 input proj
KI1 = 2 * H // 128   # 8 contraction chunks, layer-1 input proj
TBLK = 8             # S-phase y writeback block (steps)
NXCH = 16            # number of exchange chunks
XCH = SB // NXCH     # exchange chunk width (cols)

RING = 3             # gx ring depth (chunks)
LOOKA = 2            # fill lookahead (chunks)
CSTEP = 32           # steps per gx chunk
CCOL = CSTEP * BQ    # 512 cols per gx chunk
NCH = S // CSTEP     # 16 chunks


def _scan(ctx, tc, nc, layer, whhT_dram, nbias_dram, ident_dram, gb_dram,
          wih_sb, ki, rhs_fn, y0own, y1T_dram, y0ex_dram):
    """512-step GRU scan with interleaved gx production.

    rhs_fn(c, st) -> (pre_actions, rhs_getters): emission-time callbacks
    producing the ki moving operands [128, CCOL] for gx chunk c.
    """
    nc_ = nc
    tag = f"s{layer}"
    wpool = ctx.enter_context(tc.tile_pool(name=f"whh_{tag}", bufs=1))
    cpool = ctx.enter_context(tc.tile_pool(name=f"c_{tag}", bufs=1))
    rpool = ctx.enter_context(tc.tile_pool(name=f"ring_{tag}", bufs=1))
    psum = ctx.enter_context(tc.tile_pool(name=f"ps_{tag}", bufs=1, space="PSUM"))
    fps = ctx.enter_context(tc.tile_pool(name=f"fps_{tag}", bufs=2, space="PSUM"))
    dpool = ctx.enter_context(tc.tile_pool(name=f"dum_{tag}", bufs=1, space="PSUM"))
    gp = ctx.enter_context(tc.tile_pool(name=f"g_{tag}", bufs=3))
    yp = ctx.enter_context(tc.tile_pool(name=f"y_{tag}", bufs=3))

    whh = wpool.tile([128, F, G], BF16)
    nc_.sync.dma_start(whh[:], whhT_dram.ap().rearrange("(k p) g -> p k g", p=128))
    ident = cpool.tile([128, 128], BF16)
    nc_.sync.dma_start(ident[:], ident_dram.ap())
    nbx = cpool.tile([128, F, BQ], BF16)
    nc_.sync.dma_start(nbx[:], nbias_dram.ap().rearrange("p (f b) -> p f b", b=BQ))
    gb = cpool.tile([128, M12], F32)
    nc_.sync.dma_start(gb[:], gb_dram.ap())
    zero_bf = cpool.tile([128, F, BQ], BF16)
    nc_.vector.memset(zero_bf[:], 0.0)

    ring = rpool.tile([128, M12, RING * CCOL], BF16)

    # HAM-bridge: dummy matmuls into an unread PSUM bank fill the PE-idle
    # gap between a step's burst+fills and the next burst (gated on h).
    # Sized to fit within the measured cold gap (S0 ~1.4us, S1 ~0.45us),
    # so they cost nothing if the clock stays throttled, but keep the
    # activity monitor fed so the PE can hold 2.4 GHz when it unthrottles.
    dum_src = cpool.tile([128, 512], BF16)
    nc_.vector.memset(dum_src[:], 0.0)
    NDUM = 3 if layer == 0 else 1

    def dummies():
        for _ in range(NDUM):
            dps = dpool.tile([128, 512], F32, tag="dum", name="dum_ps")
            nc_.tensor.matmul(dps[:], lhsT=ident[:], rhs=dum_src[:],
                              start=True, stop=True, skip_group_check=True)

    # ---- gx production actions (fills) ----
    def chunk_actions(c):
        acts = []
        st = {}
        pre, rhs_get = rhs_fn(c, st)
        acts.extend(pre)
        slot = c % RING

        def mk_mm(m, k):
            def act():
                if k == 0:
                    st[("ps", m)] = fps.tile([128, CCOL], F32, tag="fill",
                                             name=f"fill_ps_{tag}")
                nc_.tensor.matmul(
                    st[("ps", m)][:],
                    lhsT=wih_sb[:, k, ts(m, 128)],
                    rhs=rhs_get[k](c),
                    start=(k == 0),
                    stop=(k == ki - 1),
                    skip_group_check=True,
                )
            return act

        def mk_copy(m):
            def act():
                ps = st.pop(("ps", m))
                dst = ring[:, m, slot * CCOL:(slot + 1) * CCOL]
                if m % 2 == 0:
                    nc_.scalar.activation(dst, ps[:], AF.Identity,
                                          bias=gb[:, m:m + 1])
                else:
                    nc_.vector.tensor_scalar_add(dst, ps[:], gb[:, m:m + 1])
            return act

        for m in range(M12):
            for k in range(ki):
                acts.append(mk_mm(m, k))
            acts.append(mk_copy(m))
        return acts

    # bootstrap only chunk 0; chunk 1 streams at double rate into the
    # first 16 steps (ring depth 3 keeps its slot free), chunks 2+ are
    # produced across the 32 steps of chunk c-2 as before
    for a in chunk_actions(0):
        a()
    fills = []
    for c in range(1, NCH):
        acts = chunk_actions(c)
        if c == 1:
            base, span = 0, CSTEP // 2
        else:
            base, span = (c - LOOKA) * CSTEP, CSTEP
        n = len(acts)
        for i, a in enumerate(acts):
            fills.append((base + (i * span) // n, a))
    fills.sort(key=lambda x: x[0])  # stable: keeps per-chunk action order
    fptr = 0

    y1_r = None
    if y1T_dram is not None:
        y1_r = y1T_dram.ap().rearrange("(f p) c -> p f c", p=128)

    h_prev = zero_bf[:]
    y1sb = None
    for u in range(S):
        j = u % TBLK
        if j == 0 and layer == 1:
            y1sb = yp.tile([128, F, TBLK * BQ], BF16, tag="y1sb")
        off = (u // CSTEP % RING) * CCOL + (u % CSTEP) * BQ
        col = ts(j, BQ)
        ghr = psum.tile([128, F, BQ], F32, tag="ghr")
        ghn = psum.tile([128, F, BQ], F32, tag="ghn")
        ghz = psum.tile([128, F, BQ], F32, tag="ghz")

        def mm_group(gate, ps):
            m0 = {"r": 0, "z": F, "n": 2 * F}[gate]
            inj = nbx[:] if gate == "n" else ring[:, m0:m0 + F, off:off + BQ]
            nc_.tensor.matmul(ps[:], lhsT=ident[:], rhs=inj,
                              start=True, stop=False, skip_group_check=True)
            for f in range(F):
                m = m0 + f
                for k in range(F):
                    nc_.tensor.matmul(ps[:, f, :], lhsT=whh[:, k, ts(m, 128)],
                                      rhs=h_prev[:, k, :],
                                      start=False, stop=(f == F - 1 and k == F - 1),
                                      skip_group_check=True)

        # ---- PE burst: r group, n group, z group ----
        mm_group("r", ghr)
        r = gp.tile([128, F, BQ], F32, tag="r")
        nc_.scalar.activation(r[:], ghr[:], AF.Sigmoid)

        mm_group("n", ghn)
        tn = gp.tile([128, F, BQ], F32, tag="tn")
        nc_.vector.tensor_tensor(tn[:], r[:], ghn[:], ALU.mult)
        tn2 = gp.tile([128, F, BQ], F32, tag="tn2")
        nc_.vector.tensor_tensor(tn2[:], tn[:], ring[:, 2 * F:3 * F, off:off + BQ],
                                 ALU.add)

        mm_group("z", ghz)
        z = gp.tile([128, F, BQ], F32, tag="z")
        nc_.scalar.activation(z[:], ghz[:], AF.Sigmoid)
        n = gp.tile([128, F, BQ], F32, tag="n")
        nc_.scalar.activation(n[:], tn2[:], AF.Tanh)

        p1 = gp.tile([128, F, BQ], F32, tag="p1")
        nc_.gpsimd.tensor_tensor(p1[:], z[:], h_prev, ALU.mult)
        m1n = gp.tile([128, F, BQ], F32, tag="m1n")
        nc_.vector.scalar_tensor_tensor(m1n[:], z[:], 1.0, n[:],
                                        ALU.subtract, ALU.mult)

        if layer == 0:
            hslot = y0own[:, :, ts(u, BQ)]
        else:
            hslot = y1sb[:, :, col]
        nc_.vector.tensor_tensor(hslot, p1[:], m1n[:], ALU.subtract)
        h_prev = hslot

        # ---- fills: interleaved gx production for chunk u//CSTEP + LOOKA ----
        while fptr < len(fills) and fills[fptr][0] <= u:
            fills[fptr][1]()
            fptr += 1
        dummies()

        if j == TBLK - 1:
            blk = u // TBLK
            if layer == 0:
                # mirror this block of h states to y0ex, time-reversed at
                # BQ-block granularity (partner processing order).  y0ex is
                # [NXCH, H, XCH] so each exchange chunk is contiguous.
                y0e = y0ex_dram.ap()
                c0 = (S - 1 - blk * TBLK) * BQ   # start col (partner order)
                xj = c0 // XCH                   # exchange chunk index
                for f in range(F):
                    dst = bass.AP(
                        tensor=y0e.tensor,
                        offset=xj * H * XCH + f * 128 * XCH + (c0 - xj * XCH),
                        ap=[[XCH, 128], [-BQ, TBLK], [1, BQ]],
                    )
                    src = y0own[:, f, ts(blk, TBLK * BQ)].rearrange(
                        "p (t b) -> p t b", b=BQ)
                    nc_.sync.dma_start(dst, src)
            else:
                nc_.sync.dma_start(y1_r[:, :, ts(blk, TBLK * BQ)], y1sb[:])


def build_program(debug=False):
    nc = bacc.Bacc("TRN2", target_bir_lowering=False, debug=debug,
                   num_devices=NCORE)

    def din(name, shape, dt):
        return nc.dram_tensor(name, list(shape), dt, kind="ExternalInput")

    xT = din("xT", (I, SB), BF16)
    wih0T = din("wih0T", (I, G), BF16)
    whh0T = din("whh0T", (H, G), BF16)
    wih1T = din("wih1T", (2 * H, G), BF16)
    whh1T = din("whh1T", (H, G), BF16)
    gbias0 = din("gbias0", (128, M12), F32)
    gbias1 = din("gbias1", (128, M12), F32)
    nbias0 = din("nbias0", (128, F * BQ), BF16)
    nbias1 = din("nbias1", (128, F * BQ), BF16)
    ident = din("ident", (128, 128), BF16)

    y1T = nc.dram_tensor("y1T", [H, SB], BF16, kind="ExternalOutput")
    y0ex = nc.dram_tensor("y0ex", [NXCH, H, XCH], BF16)
    y0g = nc.dram_tensor("y0g", [NXCH, 2, H, XCH], BF16)

    groups = [[2 * q, 2 * q + 1] for q in range(4)]

    with tile.TileContext(nc) as tc:
        with ExitStack() as ctx:
            y0pool = ctx.enter_context(tc.tile_pool(name="y0own", bufs=1))
            y0own = y0pool.tile([128, F, SB], BF16)

            # ---- layer 0: scan with interleaved input projection ----
            with ExitStack() as s0ctx:
                xpool = s0ctx.enter_context(tc.tile_pool(name="xsb", bufs=1))
                xsb = xpool.tile([128, KI0, SB], BF16)
                nc.sync.dma_start(xsb[:],
                                  xT.ap().rearrange("(k p) c -> p k c", p=128))
                w0pool = s0ctx.enter_context(tc.tile_pool(name="wih0", bufs=1))
                wih0 = w0pool.tile([128, KI0, G], BF16)
                nc.sync.dma_start(wih0[:],
                                  wih0T.ap().rearrange("(k p) g -> p k g", p=128))

                def rhs0(c, st):
                    getters = [
                        (lambda k: (lambda cc: xsb[:, k, ts(cc, CCOL)]))(k)
                        for k in range(KI0)
                    ]
                    return [], getters

                _scan(s0ctx, tc, nc, 0, whh0T, nbias0, ident, gbias0,
                      wih0, KI0, rhs0, y0own, None, y0ex)

            # ---- exchange: chunked pairwise AllGather, fired as the scan
            # completes each (high-to-low) column chunk of y0ex ----
            for j in range(NXCH - 1, -1, -1):
                nc.gpsimd.collective_compute(
                    "AllGather", ALU.bypass,
                    ins=[y0ex.ap()[j]],
                    outs=[y0g.ap()[j]],
                    replica_groups=groups,
                )

            # ---- layer 1: scan with interleaved input projection ----
            with ExitStack() as s1ctx:
                w1pool = s1ctx.enter_context(tc.tile_pool(name="wih1", bufs=1))
                wih1 = w1pool.tile([128, KI1, G], BF16)
                nc.sync.dma_start(wih1[:],
                                  wih1T.ap().rearrange("(k p) g -> p k g", p=128))
                ppool = s1ctx.enter_context(tc.tile_pool(name="part", bufs=2))
                rank = nc.sync.cc_rank(groups)

                def part_src(c, rr):
                    # XCH == CCOL: P1 chunk c maps 1:1 to exchange chunk c
                    return y0g.ap()[c, rr].rearrange(
                        "(k p) c -> p k c", p=128)

                def rhs1(c, st):
                    def stage():
                        part = ppool.tile([128, F, CCOL], BF16, tag="part",
                                          name="part_stage")
                        st["part"] = part
                        with tc.If(rank < 1) as cmp:
                            nc.sync.dma_start(part[:], part_src(c, 1))
                        with cmp.Else():
                            nc.sync.dma_start(part[:], part_src(c, 0))

                    getters = [
                        (lambda k: (lambda cc: y0own[:, k, ts(cc, CCOL)]))(k)
                        for k in range(F)
                    ] + [
                        (lambda k: (lambda cc: st["part"][:, k, :]))(k)
                        for k in range(F)
                    ]
                    return [stage], getters

                _scan(s1ctx, tc, nc, 1, whh1T, nbias1, ident, gbias1,
                      wih1, KI1, rhs1, y0own, y1T, None)

    nc.compile()
    return nc


_PROGRAM_CACHE = {}


def _get_program():
    if "nc" not in _PROGRAM_CACHE:
        _PROGRAM_CACHE["nc"] = build_program()
    return _PROGRAM_CACHE["nc"]


def _host_inputs(inputs):
    """Build the 8 per-core input maps from the full problem inputs."""
    bf = ml_dtypes.bfloat16
    x = np.asarray(inputs["input"], np.float32)            # (S, B, I)
    in_maps = []
    for c in range(NCORE):
        fwd = c % 2 == 0
        q = c // 2
        d = "f" if fwd else "b"
        xq = x[:, q * BQ:(q + 1) * BQ, :]
        if not fwd:
            xq = xq[::-1]
        xTv = np.ascontiguousarray(xq.transpose(2, 0, 1).reshape(I, SB))

        def wT(wname):
            return np.ascontiguousarray(np.asarray(inputs[wname], np.float32).T)

        wih0 = wT(f"Wih_{d}0")        # (I, G)
        whh0 = wT(f"Whh_{d}0")        # (H, G)
        wih1_full = wT(f"Wih_{d}1")   # (2H, G); rows = y0 features [hf | hb]
        own_sl = slice(0, H) if fwd else slice(H, 2 * H)
        par_sl = slice(H, 2 * H) if fwd else slice(0, H)
        wih1 = np.concatenate([wih1_full[own_sl], wih1_full[par_sl]], axis=0)
        whh1 = wT(f"Whh_{d}1")

        def gbias(layer):
            bih = np.asarray(inputs[f"bih_{d}{layer}"], np.float32)
            bhh = np.asarray(inputs[f"bhh_{d}{layer}"], np.float32)
            gb = np.concatenate([bih[:2 * H] + bhh[:2 * H], bih[2 * H:]])
            return np.ascontiguousarray(gb.reshape(M12, 128).T)  # [128, M12]

        def nbias(layer):
            bhh = np.asarray(inputs[f"bhh_{d}{layer}"], np.float32)
            nb = bhh[2 * H:].reshape(F, 128).T  # [128, F]
            return np.ascontiguousarray(
                np.broadcast_to(nb[:, :, None], (128, F, BQ)).reshape(
                    128, F * BQ)).astype(bf)

        in_maps.append({
            "xT": xTv.astype(bf),
            "wih0T": wih0.astype(bf), "whh0T": whh0.astype(bf),
            "wih1T": wih1.astype(bf), "whh1T": whh1.astype(bf),
            "gbias0": gbias(0), "gbias1": gbias(1),
            "nbias0": nbias(0), "nbias1": nbias(1),
            "ident": np.eye(128, dtype=bf),
        })
    return in_maps


def kernel(**inputs) -> np.ndarray:
    nc = _get_program()
    in_maps = _host_inputs(inputs)
    trace = bool(int(os.environ.get("BIGRU_TRACE", "0")))
    kw = {}
    if trace and os.environ.get("BIGRU_TRACE_DIR"):
        kw["tmpdir"] = os.environ["BIGRU_TRACE_DIR"]
    res = run_bass_kernel_spmd(nc, in_maps, list(range(NCORE)), trace=trace, **kw)
    if trace and res.exec_time_ns is not None:
        print(f"HW exec time: {res.exec_time_ns} ns")
        _PROGRAM_CACHE["exec_time_ns"] = res.exec_time_ns
        _PROGRAM_CACHE["profile_json"] = res.profile_json

    out = np.empty((S, B, 2 * H), np.float32)
    for c in range(NCORE):
        fwd = c % 2 == 0
        q = c // 2
        y = np.asarray(res.results[c]["y1T"], dtype=np.float32)
        y = y.reshape(H, S, BQ).transpose(1, 2, 0)  # (S, BQ, H)
        if not fwd:
            y = y[::-1]
        out[:, q * BQ:(q + 1) * BQ, (0 if fwd else H):(H if fwd else 2 * H)] = y
    return out


# revision 28
# speedup vs baseline: 1.0952x; 1.0952x over previous
"""BiGRU (S=512, B=64, I=256, H=512, L=2) Trainium2 Bass kernel (v2b).

Strategy: 4-way batch split x 2-way direction split across 8 NeuronCores.
Cores 0-3 run the forward GRU chain for batch quarters 0-3; cores 4-7 the
backward chain (fed time-reversed input, so the device program is
identical on every core).

v2b: the input projections (P phases) no longer run as separate phases
writing gx to DRAM.  Instead gx lives in an SBUF ring of RING chunks
(32 steps each) and the projection matmuls for chunk c+2 are interleaved
into the scan steps of chunk c ("fills"), executing in the PE-idle gap
while the per-step gate chain runs.  This keeps the tensor engine
continuously busy (so the HAM clock un-throttles 1.2 -> 2.4 GHz) and
removes the gx DRAM round trip.

Scan step schedule: PSUM groups in order r, n, z so the long n-gate
consumer chain (tn -> tn2 -> tanh -> blend) starts right at the end of
the matmul burst.  Blend uses scalar_tensor_tensor:
  h' = p1 - (z-1)*n  ==  z*h_prev + (1-z)*n.

Layer exchange: chunked pairwise AllGathers fired while S0 still runs
(y0ex fills from high columns down, so high chunks gather early); the
layer-1 fills read the partner half straight out of the gathered buffer
under an If/Else on pair rank.  Final un-transpose / un-reverse of the
output happens on the host.
"""

import os
import sys
import numpy as np

for _p in ("/opt/trn_rl_repo", "/root/.axon_site/_ro/trn_rl_repo"):
    if os.path.isdir(_p) and _p not in sys.path:
        sys.path.insert(0, _p)

import ml_dtypes
from contextlib import ExitStack

import concourse.bass as bass
import concourse.tile as tile
from concourse import bacc, mybir
from concourse.bass import ts
from concourse.bass_utils import run_bass_kernel_spmd

BF16 = mybir.dt.bfloat16
F32 = mybir.dt.float32
AF = mybir.ActivationFunctionType
ALU = mybir.AluOpType

S, B, I, H, L = 512, 64, 256, 512, 2
G = 3 * H            # 1536 gate rows (r, z, n)
NCORE = 8
BQ = B // 4          # 16 batch per core
SB = S * BQ          # 8192 moving columns
F = H // 128         # 4 h-fold chunks
M12 = G // 128       # 12 gate chunks
KI0 = I // 128       # 2 contraction chunks, layer-0 input proj
KI1 = 2 * H // 128   # 8 contraction chunks, layer-1 input proj
TBLK = 8             # S-phase y writeback block (steps)
NXCH = 16            # number of exchange chunks
XCH = SB // NXCH     # exchange chunk width (cols)

RING = 3             # gx ring depth (chunks)
LOOKA = 2            # fill lookahead (chunks)
CSTEP = 32           # steps per gx chunk
CCOL = CSTEP * BQ    # 512 cols per gx chunk
NCH = S // CSTEP     # 16 chunks


def _scan(ctx, tc, nc, layer, whhT_dram, nbias_dram, ident_dram, gb_dram,
          wih_sb, ki, rhs_fn, y0own, y1T_dram, y0ex_dram):
    """512-step GRU scan with interleaved gx production.

    rhs_fn(c, st) -> (pre_actions, rhs_getters): emission-time callbacks
    producing the ki moving operands [128, CCOL] for gx chunk c.
    """
    nc_ = nc
    tag = f"s{layer}"
    wpool = ctx.enter_context(tc.tile_pool(name=f"whh_{tag}", bufs=1))
    cpool = ctx.enter_context(tc.tile_pool(name=f"c_{tag}", bufs=1))
    rpool = ctx.enter_context(tc.tile_pool(name=f"ring_{tag}", bufs=1))
    psum = ctx.enter_context(tc.tile_pool(name=f"ps_{tag}", bufs=2, space="PSUM"))
    fps = ctx.enter_context(tc.tile_pool(name=f"fps_{tag}", bufs=2, space="PSUM"))
    gp = ctx.enter_context(tc.tile_pool(name=f"g_{tag}", bufs=3))
    yp = ctx.enter_context(tc.tile_pool(name=f"y_{tag}", bufs=3))

    whh = wpool.tile([128, F, G], BF16)
    nc_.sync.dma_start(whh[:], whhT_dram.ap().rearrange("(k p) g -> p k g", p=128))
    ident = cpool.tile([128, 128], BF16)
    nc_.sync.dma_start(ident[:], ident_dram.ap())
    nbx = cpool.tile([128, F, BQ], BF16)
    nc_.sync.dma_start(nbx[:], nbias_dram.ap().rearrange("p (f b) -> p f b", b=BQ))
    gb = cpool.tile([128, M12], F32)
    nc_.sync.dma_start(gb[:], gb_dram.ap())
    zero_bf = cpool.tile([128, F, BQ], BF16)
    nc_.vector.memset(zero_bf[:], 0.0)

    ring = rpool.tile([128, M12, RING * CCOL], BF16)

    # ---- gx production actions (fills) ----
    def chunk_actions(c):
        acts = []
        st = {}
        pre, rhs_get = rhs_fn(c, st)
        acts.extend(pre)
        slot = c % RING

        def mk_mm(m, k):
            def act():
                if k == 0:
                    st[("ps", m)] = fps.tile([128, CCOL], F32, tag="fill",
                                             name=f"fill_ps_{tag}")
                nc_.tensor.matmul(
                    st[("ps", m)][:],
                    lhsT=wih_sb[:, k, ts(m, 128)],
                    rhs=rhs_get[k](c),
                    start=(k == 0),
                    stop=(k == ki - 1),
                    skip_group_check=True,
                )
            return act

        def mk_copy(m):
            def act():
                ps = st.pop(("ps", m))
                dst = ring[:, m, slot * CCOL:(slot + 1) * CCOL]
                if m % 2 == 0:
                    nc_.scalar.activation(dst, ps[:], AF.Identity,
                                          bias=gb[:, m:m + 1])
                else:
                    nc_.vector.tensor_scalar_add(dst, ps[:], gb[:, m:m + 1])
            return act

        for m in range(M12):
            for k in range(ki):
                acts.append(mk_mm(m, k))
            acts.append(mk_copy(m))
        return acts

    # bootstrap only chunk 0; chunk 1 streams at double rate into the
    # first 16 steps (ring depth 3 keeps its slot free), chunks 2+ are
    # produced across the 32 steps of chunk c-2 as before
    for a in chunk_actions(0):
        a()
    fills = []
    for c in range(1, NCH):
        acts = chunk_actions(c)
        if c == 1:
            base, span = 0, CSTEP // 2
        else:
            base, span = (c - LOOKA) * CSTEP, CSTEP
        n = len(acts)
        for i, a in enumerate(acts):
            fills.append((base + (i * span) // n, a))
    fills.sort(key=lambda x: x[0])  # stable: keeps per-chunk action order
    fptr = 0

    y1_r = None
    if y1T_dram is not None:
        y1_r = y1T_dram.ap().rearrange("(f p) c -> p f c", p=128)

    h_prev = zero_bf[:]
    y1sb = None
    for u in range(S):
        j = u % TBLK
        if j == 0 and layer == 1:
            y1sb = yp.tile([128, F, TBLK * BQ], BF16, tag="y1sb")
        off = (u // CSTEP % RING) * CCOL + (u % CSTEP) * BQ
        col = ts(j, BQ)
        # ghr and ghz share one PSUM bank; their gx slices are adjacent
        # (m 0..3 = r, 4..7 = z) so ONE identity matmul injects both
        grz = psum.tile([128, 2 * F, BQ], F32, tag="grz")
        ghn = psum.tile([128, F, BQ], F32, tag="ghn")
        ghr = grz[:, 0:F, :]
        ghz = grz[:, F:2 * F, :]
        nc_.tensor.matmul(grz[:], lhsT=ident[:],
                          rhs=ring[:, 0:2 * F, off:off + BQ],
                          start=True, stop=False, skip_group_check=True)

        def mm_group(gate, ps):
            m0 = {"r": 0, "z": F, "n": 2 * F}[gate]
            if gate == "n":
                nc_.tensor.matmul(ps[:], lhsT=ident[:], rhs=nbx[:],
                                  start=True, stop=False, skip_group_check=True)
            for f in range(F):
                m = m0 + f
                for k in range(F):
                    nc_.tensor.matmul(ps[:, f, :], lhsT=whh[:, k, ts(m, 128)],
                                      rhs=h_prev[:, k, :],
                                      start=False, stop=(f == F - 1 and k == F - 1),
                                      skip_group_check=True)

        # ---- PE burst: r group, n group, z group ----
        mm_group("r", ghr)
        r = gp.tile([128, F, BQ], F32, tag="r")
        nc_.scalar.activation(r[:], ghr[:], AF.Sigmoid)

        mm_group("n", ghn)
        tn = gp.tile([128, F, BQ], F32, tag="tn")
        nc_.vector.tensor_tensor(tn[:], r[:], ghn[:], ALU.mult)
        tn2 = gp.tile([128, F, BQ], F32, tag="tn2")
        nc_.vector.tensor_tensor(tn2[:], tn[:], ring[:, 2 * F:3 * F, off:off + BQ],
                                 ALU.add)

        mm_group("z", ghz)
        z = gp.tile([128, F, BQ], F32, tag="z")
        nc_.scalar.activation(z[:], ghz[:], AF.Sigmoid)
        n = gp.tile([128, F, BQ], F32, tag="n")
        nc_.scalar.activation(n[:], tn2[:], AF.Tanh)

        p1 = gp.tile([128, F, BQ], F32, tag="p1")
        nc_.gpsimd.tensor_tensor(p1[:], z[:], h_prev, ALU.mult)
        m1n = gp.tile([128, F, BQ], F32, tag="m1n")
        nc_.vector.scalar_tensor_tensor(m1n[:], z[:], 1.0, n[:],
                                        ALU.subtract, ALU.mult)

        if layer == 0:
            hslot = y0own[:, :, ts(u, BQ)]
        else:
            hslot = y1sb[:, :, col]
        nc_.vector.tensor_tensor(hslot, p1[:], m1n[:], ALU.subtract)
        h_prev = hslot

        # ---- fills: interleaved gx production for chunk u//CSTEP + LOOKA ----
        while fptr < len(fills) and fills[fptr][0] <= u:
            fills[fptr][1]()
            fptr += 1

        if j == TBLK - 1:
            blk = u // TBLK
            if layer == 0:
                # mirror this block of h states to y0ex, time-reversed at
                # BQ-block granularity (partner processing order).  y0ex is
                # [NXCH, H, XCH] so each exchange chunk is contiguous.
                y0e = y0ex_dram.ap()
                c0 = (S - 1 - blk * TBLK) * BQ   # start col (partner order)
                xj = c0 // XCH                   # exchange chunk index
                for f in range(F):
                    dst = bass.AP(
                        tensor=y0e.tensor,
                        offset=xj * H * XCH + f * 128 * XCH + (c0 - xj * XCH),
                        ap=[[XCH, 128], [-BQ, TBLK], [1, BQ]],
                    )
                    src = y0own[:, f, ts(blk, TBLK * BQ)].rearrange(
                        "p (t b) -> p t b", b=BQ)
                    nc_.sync.dma_start(dst, src)
            else:
                nc_.sync.dma_start(y1_r[:, :, ts(blk, TBLK * BQ)], y1sb[:])


def build_program(debug=False):
    nc = bacc.Bacc("TRN2", target_bir_lowering=False, debug=debug,
                   num_devices=NCORE)

    def din(name, shape, dt):
        return nc.dram_tensor(name, list(shape), dt, kind="ExternalInput")

    xT = din("xT", (I, SB), BF16)
    wih0T = din("wih0T", (I, G), BF16)
    whh0T = din("whh0T", (H, G), BF16)
    wih1T = din("wih1T", (2 * H, G), BF16)
    whh1T = din("whh1T", (H, G), BF16)
    gbias0 = din("gbias0", (128, M12), F32)
    gbias1 = din("gbias1", (128, M12), F32)
    nbias0 = din("nbias0", (128, F * BQ), BF16)
    nbias1 = din("nbias1", (128, F * BQ), BF16)
    ident = din("ident", (128, 128), BF16)

    y1T = nc.dram_tensor("y1T", [H, SB], BF16, kind="ExternalOutput")
    y0ex = nc.dram_tensor("y0ex", [NXCH, H, XCH], BF16)
    y0g = nc.dram_tensor("y0g", [NXCH, 2, H, XCH], BF16)

    groups = [[2 * q, 2 * q + 1] for q in range(4)]

    with tile.TileContext(nc) as tc:
        with ExitStack() as ctx:
            y0pool = ctx.enter_context(tc.tile_pool(name="y0own", bufs=1))
            y0own = y0pool.tile([128, F, SB], BF16)

            # ---- layer 0: scan with interleaved input projection ----
            with ExitStack() as s0ctx:
                xpool = s0ctx.enter_context(tc.tile_pool(name="xsb", bufs=1))
                xsb = xpool.tile([128, KI0, SB], BF16)
                x_r = xT.ap().rearrange("(k p) c -> p k c", p=128)
                # first chunk separately so the chunk-0 bootstrap starts
                # without waiting for the full 4MB input load
                nc.sync.dma_start(xsb[:, :, 0:CCOL], x_r[:, :, 0:CCOL])
                nc.sync.dma_start(xsb[:, :, CCOL:], x_r[:, :, CCOL:])
                w0pool = s0ctx.enter_context(tc.tile_pool(name="wih0", bufs=1))
                wih0 = w0pool.tile([128, KI0, G], BF16)
                nc.sync.dma_start(wih0[:],
                                  wih0T.ap().rearrange("(k p) g -> p k g", p=128))

                def rhs0(c, st):
                    getters = [
                        (lambda k: (lambda cc: xsb[:, k, ts(cc, CCOL)]))(k)
                        for k in range(KI0)
                    ]
                    return [], getters

                _scan(s0ctx, tc, nc, 0, whh0T, nbias0, ident, gbias0,
                      wih0, KI0, rhs0, y0own, None, y0ex)

            # ---- exchange: chunked pairwise AllGather, fired as the scan
            # completes each (high-to-low) column chunk of y0ex ----
            for j in range(NXCH - 1, -1, -1):
                nc.gpsimd.collective_compute(
                    "AllGather", ALU.bypass,
                    ins=[y0ex.ap()[j]],
                    outs=[y0g.ap()[j]],
                    replica_groups=groups,
                )

            # ---- layer 1: scan with interleaved input projection ----
            with ExitStack() as s1ctx:
                w1pool = s1ctx.enter_context(tc.tile_pool(name="wih1", bufs=1))
                wih1 = w1pool.tile([128, KI1, G], BF16)
                w1_r = wih1T.ap().rearrange("(k p) g -> p k g", p=128)
                # per-k slices: the bootstrap's k=0 matmuls start after the
                # first 384KB instead of the full 3MB weight load
                for k in range(KI1):
                    nc.sync.dma_start(wih1[:, k, :], w1_r[:, k, :])
                ppool = s1ctx.enter_context(tc.tile_pool(name="part", bufs=2))
                rank = nc.sync.cc_rank(groups)

                def part_src(c, rr):
                    # XCH == CCOL: P1 chunk c maps 1:1 to exchange chunk c
                    return y0g.ap()[c, rr].rearrange(
                        "(k p) c -> p k c", p=128)

                def rhs1(c, st):
                    def stage():
                        part = ppool.tile([128, F, CCOL], BF16, tag="part",
                                          name="part_stage")
                        st["part"] = part
                        with tc.If(rank < 1) as cmp:
                            nc.sync.dma_start(part[:], part_src(c, 1))
                        with cmp.Else():
                            nc.sync.dma_start(part[:], part_src(c, 0))

                    getters = [
                        (lambda k: (lambda cc: y0own[:, k, ts(cc, CCOL)]))(k)
                        for k in range(F)
                    ] + [
                        (lambda k: (lambda cc: st["part"][:, k, :]))(k)
                        for k in range(F)
                    ]
                    return [stage], getters

                _scan(s1ctx, tc, nc, 1, whh1T, nbias1, ident, gbias1,
                      wih1, KI1, rhs1, y0own, y1T, None)

    nc.compile()
    return nc


_PROGRAM_CACHE = {}


def _get_program():
    if "nc" not in _PROGRAM_CACHE:
        _PROGRAM_CACHE["nc"] = build_program()
    return _PROGRAM_CACHE["nc"]


def _host_inputs(inputs):
    """Build the 8 per-core input maps from the full problem inputs."""
    bf = ml_dtypes.bfloat16
    x = np.asarray(inputs["input"], np.float32)            # (S, B, I)
    in_maps = []
    for c in range(NCORE):
        fwd = c % 2 == 0
        q = c // 2
        d = "f" if fwd else "b"
        xq = x[:, q * BQ:(q + 1) * BQ, :]
        if not fwd:
            xq = xq[::-1]
        xTv = np.ascontiguousarray(xq.transpose(2, 0, 1).reshape(I, SB))

        def wT(wname):
            return np.ascontiguousarray(np.asarray(inputs[wname], np.float32).T)

        wih0 = wT(f"Wih_{d}0")        # (I, G)
        whh0 = wT(f"Whh_{d}0")        # (H, G)
        wih1_full = wT(f"Wih_{d}1")   # (2H, G); rows = y0 features [hf | hb]
        own_sl = slice(0, H) if fwd else slice(H, 2 * H)
        par_sl = slice(H, 2 * H) if fwd else slice(0, H)
        wih1 = np.concatenate([wih1_full[own_sl], wih1_full[par_sl]], axis=0)
        whh1 = wT(f"Whh_{d}1")

        def gbias(layer):
            bih = np.asarray(inputs[f"bih_{d}{layer}"], np.float32)
            bhh = np.asarray(inputs[f"bhh_{d}{layer}"], np.float32)
            gb = np.concatenate([bih[:2 * H] + bhh[:2 * H], bih[2 * H:]])
            return np.ascontiguousarray(gb.reshape(M12, 128).T)  # [128, M12]

        def nbias(layer):
            bhh = np.asarray(inputs[f"bhh_{d}{layer}"], np.float32)
            nb = bhh[2 * H:].reshape(F, 128).T  # [128, F]
            return np.ascontiguousarray(
                np.broadcast_to(nb[:, :, None], (128, F, BQ)).reshape(
                    128, F * BQ)).astype(bf)

        in_maps.append({
            "xT": xTv.astype(bf),
            "wih0T": wih0.astype(bf), "whh0T": whh0.astype(bf),
            "wih1T": wih1.astype(bf), "whh1T": whh1.astype(bf),
            "gbias0": gbias(0), "gbias1": gbias(1),
            "nbias0": nbias(0), "nbias1": nbias(1),
            "ident": np.eye(128, dtype=bf),
        })
    return in_maps


def kernel(**inputs) -> np.ndarray:
    nc = _get_program()
    in_maps = _host_inputs(inputs)
    trace = bool(int(os.environ.get("BIGRU_TRACE", "0")))
    kw = {}
    if trace and os.environ.get("BIGRU_TRACE_DIR"):
        kw["tmpdir"] = os.environ["BIGRU_TRACE_DIR"]
    res = run_bass_kernel_spmd(nc, in_maps, list(range(NCORE)), trace=trace, **kw)
    if trace and res.exec_time_ns is not None:
        print(f"HW exec time: {res.exec_time_ns} ns")
        _PROGRAM_CACHE["exec_time_ns"] = res.exec_time_ns
        _PROGRAM_CACHE["profile_json"] = res.profile_json

    out = np.empty((S, B, 2 * H), np.float32)
    for c in range(NCORE):
        fwd = c % 2 == 0
        q = c // 2
        y = np.asarray(res.results[c]["y1T"], dtype=np.float32)
        y = y.reshape(H, S, BQ).transpose(1, 2, 0)  # (S, BQ, H)
        if not fwd:
            y = y[::-1]
        out[:, q * BQ:(q + 1) * BQ, (0 if fwd else H):(H if fwd else 2 * H)] = y
    return out
